# revision 37
# baseline (speedup 1.0000x reference)
"""MoE top-1 routing layer on 8 Trainium2 NeuronCores (expert-parallel).

Math: out[t] = (x[t] @ W[e] + b[e]) @ OW + ob   with e = argmax(x[t] @ GW + gb).

Both device matmuls are linear, so the host pre-fuses the weights:
  M[e]     = W[e] @ OW                    [D, O]  (exact fp32, cast bf16)
  bias2[e] = b[e] @ OW + ob               [O]     (fp64, added on host)
  out[t]   = x[t] @ M[e] + bias2[e]
so the device runs a single [C, D] @ [D, O] GEMM per core -- 1/4 the
FLOPs of the unfused two-matmul chain.

Sharding: expert-parallel. Host computes the gate (fp64 -> exact argmax),
sorts tokens by expert, pads each expert's token set to capacity C=1024
(capacity factor 1.0), and ships core e: x (gathered tokens,
transposed/striped) and M[e]. Each core returns its C token outputs
(bf16); host scatters rows back and adds bias2. Overflow tokens beyond
capacity (~3% of tokens for this routing) take an exact fp32 host path.

Device layouts (one DMA trigger per tile/chunk, 1-2 KB descriptors):
  xt{t}: [128, D/128, 128]     one per 128-token tile
  m:     [2, 4, 128, 1024]     per-(column-half h, k-pair g) chunks
  out:   [C, O] bf16
The GEMM runs in two column-half passes so the first pass only waits on
1 MB of M; the second pass is pure compute overlapped with output DMA.
A warmup matmul block keeps the PE busy through the DMA ramp so the HAM
clock gate reaches 8/8 (2.4 GHz) before the real matmuls start, and the
first pass's leading tiles interleave at k-pair granularity so the PE
chases the arriving M chunks without idle gaps.
"""

import numpy as np
from contextlib import ExitStack

B, S, D, E, H, O = 4, 2048, 1024, 8, 2048, 1024
T = B * S
P = 128
KO_D = D // P     # 8
# Per-expert device capacity = 8 full 128-token tiles (capacity factor 1.0:
# C == T/E == mean expert load). A tile's matmul cost scales with the
# 512-wide moving dim, not the token count, so a 94-token tail tile would
# cost a full tile of PE time; overflow tokens (~3.3% for the graded
# routing) take the host fallback path instead.
C = 8 * P         # 1024
TILES = [(t * P, P) for t in range(8)]

MM_DT = "bf16"    # matmul operands + device output (fp32 PSUM accumulation)


def _legalize_waits(nc):
    """This container's walrus accepts 1 sem wait per instruction (2 for
    EventSemaphore); Tile's tail drain can carry more. Split the excess
    onto preceding same-engine NoOps."""
    from concourse import mybir

    uid = 0
    for f in nc.m.functions:
        for b in f.blocks:
            insts = b.instructions
            out = []
            changed = False
            for ins in insts:
                si = ins.sync_info
                waits = list(si.on_wait) if si is not None else []
                limit = 2 if str(ins.opcode) == "EventSemaphore" else 1
                if len(waits) > limit:
                    extra, keep = waits[:-limit], waits[-limit:]
                    for w in extra:
                        uid += 1
                        out.append(
                            mybir.InstNoOp(
                                name=f"waitsplit-{uid}",
                                engine=ins.engine,
                                sync_info=mybir.SyncInfo(on_wait=[w], on_update=[]),
                                bass_nofuse=True,
                            )
                        )
                    si.on_wait = keep
                    changed = True
                out.append(ins)
            if changed:
                insts.clear()
                insts.extend(out)


def _patch_tail_barrier(tile_mod):
    """Tile's kernel tail is drain -> barrier -> sem-reset -> barrier.
    The second all-engine barrier only orders the sem-reset against program
    end, which the per-engine stream end already guarantees; drop it."""
    if getattr(tile_mod.TileContext, "_moe_tail_patched", False):
        return
    from concourse.vector_clock import ScopedClock

    def _drain_and_barrier(self, tick_clock, wait_clock):
        drain_inst = self.nc.sync.drain()
        wait_clock.add_sem_waits(
            drain_inst.ins, ScopedClock({None: tick_clock.global_clock})
        )
        self.nc.all_engine_barrier()
        popped = self.nc._tile_sem_poison_stack.pop()
        assert popped is self._sem_poison
        self.nc.clear_and_free_semaphores(list(self.sems.allocated().values()))

    tile_mod.TileContext._drain_and_barrier = _drain_and_barrier
    tile_mod.TileContext._moe_tail_patched = True


WARMUP_MMS = 19
INTERLEAVED_TILES = 3  # pass-A tiles interleaved at k-pair granularity


def _emit(nc, tile, mm_dt, f32):
    """Single fused GEMM out = x @ M, x and M SBUF-resident.

    Input loads issue on the SP HWDGE ring (nc.sync), output stores on the
    ACT ring (nc.scalar): DMA_DIRECT2D costs ~0.6us of issue time on its
    sequencer, so keeping the 18 stores off the input FIFO stops them
    head-of-line blocking the x/M stream. A memset-fed warmup matmul block
    keeps the PE busy while the first M chunks stream in, flipping the HAM
    clock gate to 8/8 (2.4 GHz) before the real matmuls start.
    """
    xts = [
        nc.dram_tensor(f"xt{t}", [P, KO_D, w], mm_dt, kind="ExternalInput")
        for t, (t0, w) in enumerate(TILES)
    ]
    # m[h, g] = one DMA chunk: column-half h, k-pair g (2 k-tiles, 256 KB)
    m = nc.dram_tensor("m", [2, 4, P, 2 * 512], mm_dt, kind="ExternalInput")
    out = nc.dram_tensor("out", [C, O], mm_dt, kind="ExternalOutput")

    with tile.TileContext(nc) as tc:
        with ExitStack() as ctx:
            x_pool = ctx.enter_context(tc.tile_pool(name="x", bufs=1))
            m_pool = ctx.enter_context(tc.tile_pool(name="m", bufs=1))
            wu_pool = ctx.enter_context(tc.tile_pool(name="wu", bufs=1))
            st_pool = ctx.enter_context(tc.tile_pool(name="st", bufs=3))
            ps_pool = ctx.enter_context(
                tc.tile_pool(name="ps", bufs=4, space="PSUM")
            )
            psw_pool = ctx.enter_context(
                tc.tile_pool(name="psw", bufs=1, space="PSUM")
            )

            m_sb = m_pool.tile([P, KO_D, 2, 512], mm_dt)
            x_sbs = [
                x_pool.tile([P, KO_D, w], mm_dt, name=f"x{t}")
                for t, (t0, w) in enumerate(TILES)
            ]

            # Demand-ordered loads on the SP ring, paced so pass A's first
            # tile chases the M k-pair chunks at ~1 chunk per 0.85us of
            # compute with no PE idle gap (HAM needs ~3.4us of continuous
            # PE activity to unthrottle to 2.4 GHz): first M chunk, x tile
            # 0, rest of M's first column-half, x tiles 1-2, M's second
            # half, then the remaining x tiles (consumed 1.7us apart).
            nc.sync.dma_start(m_sb[:, 0:2, 0], m[0, 0])
            nc.sync.dma_start(x_sbs[0][:], xts[0][:])
            nc.sync.dma_start(x_sbs[1][:], xts[1][:])
            nc.sync.dma_start(x_sbs[2][:], xts[2][:])
            for g in range(1, 4):
                nc.sync.dma_start(m_sb[:, 2 * g : 2 * g + 2, 0], m[0, g])
            for t in (3, 4, 5):
                nc.sync.dma_start(x_sbs[t][:], xts[t][:])
            for g in range(4):
                nc.sync.dma_start(m_sb[:, 2 * g : 2 * g + 2, 1], m[1, g])
            for t in (6, 7):
                nc.sync.dma_start(x_sbs[t][:], xts[t][:])

            # PE warmup: garbage matmuls from a memset tile while the M
            # chunks stream in. >=3.4us of continuous PE activity = one HAM
            # SHORT window -> 2.4 GHz for the real matmuls.
            wu = wu_pool.tile([P, 256], mm_dt)
            nc.gpsimd.memset(wu[:], 0)
            ps_w = psw_pool.tile([P, 256], f32)
            for _ in range(WARMUP_MMS):
                nc.tensor.matmul(ps_w, wu[:, :128], wu[:], start=True, stop=True)

            def finish_tile(t, h, ps):
                t0, w = TILES[t]
                st = st_pool.tile([P, 512], mm_dt)
                nc.vector.tensor_copy(st[:w], ps)
                nc.scalar.dma_start(out[t0 : t0 + w, h * 512 : (h + 1) * 512], st[:w])

            def half(h, start_t):
                for t in range(start_t, len(TILES)):
                    t0, w = TILES[t]
                    if h == 1 and t == len(TILES) - 1:
                        # Final tile: two 256-col PSUM groups so the first
                        # half's cast+store overlaps the second half's
                        # matmuls, shortening the post-last-matmul tail.
                        # Separate ps tiles per group -- sharing one tile
                        # makes Tile serialize group b's matmuls behind
                        # group a's cast (write-after-read on the tile),
                        # a measured ~0.7us stall on every core. The very
                        # last store issues on the idle SP ring.
                        st = st_pool.tile([P, 512], mm_dt)
                        for hc in (0, 1):
                            sl = slice(256 * hc, 256 * hc + 256)
                            ps = ps_pool.tile([P, 512], f32, name="ps")[:, :256]
                            for k in range(KO_D):
                                nc.tensor.matmul(
                                    ps,
                                    x_sbs[t][:, k],
                                    m_sb[:, k, h, sl],
                                    start=(k == 0),
                                    stop=(k == KO_D - 1),
                                )
                            nc.vector.tensor_copy(st[:, sl], ps)
                            dma_eng = nc.scalar if hc == 0 else nc.sync
                            dma_eng.dma_start(
                                out[t0 : t0 + w, 512 + 256 * hc : 768 + 256 * hc],
                                st[:, sl],
                            )
                        continue
                    ps = ps_pool.tile([P, 512], f32, name="ps")[:w]
                    for k in range(KO_D):
                        nc.tensor.matmul(
                            ps,
                            x_sbs[t][:, k],
                            m_sb[:, k, h],
                            start=(k == 0),
                            stop=(k == KO_D - 1),
                        )
                    finish_tile(t, h, ps)

            # Pass A's first tiles interleave at k-pair granularity so each
            # arriving M chunk feeds 2*INTERLEAVED_TILES back-to-back
            # matmuls -- enough to cover the ~0.7-1.4us chunk interarrival
            # (jittery: the ramp saturates chip HBM across all 8 cores) and
            # keep the PE gapless while the first column-half streams in.
            ps_il = [
                ps_pool.tile([P, 512], f32, name="ps")
                for _ in range(INTERLEAVED_TILES)
            ]
            for g in range(4):
                for t in range(INTERLEAVED_TILES):
                    for k in (2 * g, 2 * g + 1):
                        nc.tensor.matmul(
                            ps_il[t],
                            x_sbs[t][:, k],
                            m_sb[:, k, 0],
                            start=(k == 0),
                            stop=(k == KO_D - 1),
                        )
            for t in range(INTERLEAVED_TILES):
                finish_tile(t, 0, ps_il[t])
            half(0, INTERLEAVED_TILES)
            half(1, 0)
    return nc


def _patch_walrus_policy():
    """Compile with walrus --policy=2 (heuristics post-scheduler, ~1.5us
    faster than --policy=0 here) and --enable-ldw-opt=true (keeps
    LDWEIGHTS double-buffered behind the previous matmul; without it each
    matmul pays a serialized ~50ns weight load: measured 263 vs 213 ns/MM)."""
    import os
    import concourse.bass_utils as bu

    if getattr(bu, "_moe_policy_patched", False):
        return
    orig = bu.run_command
    extra = [a for a in os.environ.get("MOE_WALRUS_EXTRA", "").split(",") if a]
    policy = os.environ.get("MOE_POLICY", "2")

    def _rc(argv, **kw):
        if argv and "walrus_driver" in str(argv[0]):
            argv = [f"--policy={policy}" if a == "--policy=0" else a for a in argv]
            argv = argv[:1] + ["--enable-ldw-opt=true"] + extra + argv[1:]
        return orig(argv, **kw)

    bu.run_command = _rc
    bu._moe_policy_patched = True


def _build_nc():
    import concourse.bass as bass
    import concourse.tile as tile
    from concourse import mybir

    _patch_tail_barrier(tile)
    _patch_walrus_policy()
    f32 = mybir.dt.float32
    mm_dt = mybir.dt.bfloat16
    nc = bass.Bass()
    _emit(nc, tile, mm_dt, f32)
    _legalize_waits(nc)
    return nc


_NC_CACHE = {}


def kernel(x, gate_w, gate_b, expert_w, expert_b, out_w, out_b):
    import os

    # The device path runs through the axon PJRT plugin; make sure a
    # harness-pinned JAX_PLATFORMS=cpu doesn't exclude it.
    plats = os.environ.get("JAX_PLATFORMS")
    if plats and "axon" not in plats:
        os.environ["JAX_PLATFORMS"] = plats + ",axon"

    from concourse.bass_utils import run_bass_kernel_spmd

    import ml_dtypes

    mm_np = ml_dtypes.bfloat16

    x = np.asarray(x, dtype=np.float32)
    gate_w = np.asarray(gate_w, dtype=np.float32)
    gate_b = np.asarray(gate_b, dtype=np.float32)
    expert_w = np.asarray(expert_w, dtype=np.float32)
    expert_b = np.asarray(expert_b, dtype=np.float32)
    out_w = np.asarray(out_w, dtype=np.float32)
    out_b = np.asarray(out_b, dtype=np.float32)

    xt = x.reshape(T, D)
    # Gate on host in fp64: argmax matches the fp32 reference exactly
    # (min top-2 logit gap is ~1e-5, fp64 error ~1e-12).
    logits = xt.astype(np.float64) @ gate_w.astype(np.float64) + gate_b.astype(
        np.float64
    )
    idx = np.argmax(logits, axis=1)

    # Fused per-expert weight M[e] = W[e] @ OW (exact fp32 on host), packed
    # into per-(column-half h, k-pair g) DMA chunks:
    # m[h, g, p, (kk*512+c)] = M[(2g+kk)*128+p, h*512+c]
    m_f32 = {}

    def pack_m(e):
        Me = expert_w[e] @ out_w
        m_f32[e] = Me
        return np.ascontiguousarray(
            Me.astype(mm_np)
            .reshape(4, 2, P, 2, 512)
            .transpose(3, 0, 2, 1, 4)
            .reshape(2, 4, P, 2 * 512)
        )

    tok_of_expert = [np.nonzero(idx == e)[0] for e in range(E)]
    in_maps = []
    kept = []
    overflow = []
    for e in range(E):
        toks = tok_of_expert[e]
        if len(toks) > C:
            overflow.append((e, toks[C:]))
            toks = toks[:C]
        kept.append(toks)
        xpad = np.zeros((D, C), dtype=mm_np)
        xpad[:, : len(toks)] = xt[toks].T.astype(mm_np)
        # xt{t}[p, k, j] = xpad[k*128+p, t0+j]
        xk = xpad.reshape(KO_D, P, C)
        im = {"m": pack_m(e)}
        for t, (t0, w) in enumerate(TILES):
            im[f"xt{t}"] = np.ascontiguousarray(
                xk[:, :, t0 : t0 + w].transpose(1, 0, 2)
            )
        in_maps.append(im)

    if "nc" not in _NC_CACHE:
        _NC_CACHE["nc"] = _build_nc()
    nc = _NC_CACHE["nc"]

    res = run_bass_kernel_spmd(nc, in_maps, list(range(E)))

    bias2 = (
        expert_b.astype(np.float64) @ out_w.astype(np.float64)
        + out_b.astype(np.float64)
    ).astype(np.float32)  # [E, O]

    out = np.empty((T, O), dtype=np.float32)
    for e in range(E):
        toks = kept[e]
        out[toks] = res.results[e]["out"][: len(toks)].astype(np.float32) + bias2[e]
    for e, toks in overflow:
        out[toks] = xt[toks] @ m_f32[e] + bias2[e]
    return out.reshape(B, S, O)


# revision 38
# speedup vs baseline: 1.2021x; 1.2021x over previous
"""MoE top-1 routing layer on 8 Trainium2 NeuronCores (expert-parallel).

Math: out[t] = (x[t] @ W[e] + b[e]) @ OW + ob   with e = argmax(x[t] @ GW + gb).

Both device matmuls are linear, so the host pre-fuses the weights:
  M[e]     = W[e] @ OW                    [D, O]  (exact fp32, cast bf16)
  bias2[e] = b[e] @ OW + ob               [O]     (fp64, added on host)
  out[t]   = x[t] @ M[e] + bias2[e]
so the device runs a single [C, D] @ [D, O] GEMM per core -- 1/4 the
FLOPs of the unfused two-matmul chain.

Sharding: expert-parallel. Host computes the gate (fp64 -> exact argmax),
sorts tokens by expert, pads each expert's token set to capacity C=1024
(capacity factor 1.0), and ships core e: x (gathered tokens,
transposed/striped) and M[e]. Each core returns its C token outputs
(bf16); host scatters rows back and adds bias2. Overflow tokens beyond
capacity (~3% of tokens for this routing) take an exact fp32 host path.

Device layouts (one DMA trigger per tile/chunk, 1-2 KB descriptors):
  xt{t}: [128, D/128, 128]     one per 128-token tile
  m:     [2, 4, 128, 1024]     per-(column-half h, k-pair g) chunks
  out:   [C, O] bf16
The GEMM runs in two column-half passes so the first pass only waits on
1 MB of M; the second pass is pure compute overlapped with output DMA.
A warmup matmul block keeps the PE busy through the DMA ramp so the HAM
clock gate reaches 8/8 (2.4 GHz) before the real matmuls start, and the
first pass's leading tiles interleave at k-pair granularity so the PE
chases the arriving M chunks without idle gaps.
"""

import numpy as np
from contextlib import ExitStack

B, S, D, E, H, O = 4, 2048, 1024, 8, 2048, 1024
T = B * S
P = 128
KO_D = D // P     # 8
# Per-expert device capacity = 8 full 128-token tiles (capacity factor 1.0:
# C == T/E == mean expert load). A tile's matmul cost scales with the
# 512-wide moving dim, not the token count, so a 94-token tail tile would
# cost a full tile of PE time; overflow tokens (~3.3% for the graded
# routing) take the host fallback path instead.
C = 8 * P         # 1024
TILES = [(t * P, P) for t in range(8)]

MM_DT = "bf16"    # matmul operands + device output (fp32 PSUM accumulation)


def _legalize_waits(nc):
    """This container's walrus accepts 1 sem wait per instruction (2 for
    EventSemaphore); Tile's tail drain can carry more. Split the excess
    onto preceding same-engine NoOps."""
    from concourse import mybir

    uid = 0
    for f in nc.m.functions:
        for b in f.blocks:
            insts = b.instructions
            out = []
            changed = False
            for ins in insts:
                si = ins.sync_info
                waits = list(si.on_wait) if si is not None else []
                limit = 2 if str(ins.opcode) == "EventSemaphore" else 1
                if len(waits) > limit:
                    extra, keep = waits[:-limit], waits[-limit:]
                    for w in extra:
                        uid += 1
                        out.append(
                            mybir.InstNoOp(
                                name=f"waitsplit-{uid}",
                                engine=ins.engine,
                                sync_info=mybir.SyncInfo(on_wait=[w], on_update=[]),
                                bass_nofuse=True,
                            )
                        )
                    si.on_wait = keep
                    changed = True
                out.append(ins)
            if changed:
                insts.clear()
                insts.extend(out)


def _patch_tail_barrier(tile_mod):
    """Tile's kernel tail is drain -> barrier -> sem-reset -> barrier.
    The second all-engine barrier only orders the sem-reset against program
    end, which the per-engine stream end already guarantees; drop it."""
    if getattr(tile_mod.TileContext, "_moe_tail_patched", False):
        return
    from concourse.vector_clock import ScopedClock

    def _drain_and_barrier(self, tick_clock, wait_clock):
        drain_inst = self.nc.sync.drain()
        wait_clock.add_sem_waits(
            drain_inst.ins, ScopedClock({None: tick_clock.global_clock})
        )
        self.nc.all_engine_barrier()
        popped = self.nc._tile_sem_poison_stack.pop()
        assert popped is self._sem_poison
        self.nc.clear_and_free_semaphores(list(self.sems.allocated().values()))

    tile_mod.TileContext._drain_and_barrier = _drain_and_barrier
    tile_mod.TileContext._moe_tail_patched = True


WARMUP_MMS = 19
INTERLEAVED_TILES = 3  # pass-A tiles interleaved at k-pair granularity


def _emit(nc, tile, mm_dt, f32):
    """Single fused GEMM out = x @ M, x and M SBUF-resident.

    Input loads issue on the SP HWDGE ring (nc.sync), output stores on the
    ACT ring (nc.scalar): DMA_DIRECT2D costs ~0.6us of issue time on its
    sequencer, so keeping the 18 stores off the input FIFO stops them
    head-of-line blocking the x/M stream. A memset-fed warmup matmul block
    keeps the PE busy while the first M chunks stream in, flipping the HAM
    clock gate to 8/8 (2.4 GHz) before the real matmuls start.
    """
    xts = [
        nc.dram_tensor(f"xt{t}", [P, KO_D, w], mm_dt, kind="ExternalInput")
        for t, (t0, w) in enumerate(TILES)
    ]
    # m[h, g] = one DMA chunk: column-half h, k-pair g (2 k-tiles, 256 KB)
    m = nc.dram_tensor("m", [2, 4, P, 2 * 512], mm_dt, kind="ExternalInput")
    out = nc.dram_tensor("out", [C, O], mm_dt, kind="ExternalOutput")

    with tile.TileContext(nc) as tc:
        with ExitStack() as ctx:
            x_pool = ctx.enter_context(tc.tile_pool(name="x", bufs=1))
            m_pool = ctx.enter_context(tc.tile_pool(name="m", bufs=1))
            wu_pool = ctx.enter_context(tc.tile_pool(name="wu", bufs=1))
            st_pool = ctx.enter_context(tc.tile_pool(name="st", bufs=3))
            ps_pool = ctx.enter_context(
                tc.tile_pool(name="ps", bufs=4, space="PSUM")
            )
            psw_pool = ctx.enter_context(
                tc.tile_pool(name="psw", bufs=1, space="PSUM")
            )

            m_sb = m_pool.tile([P, KO_D, 2, 512], mm_dt)
            x_sbs = [
                x_pool.tile([P, KO_D, w], mm_dt, name=f"x{t}")
                for t, (t0, w) in enumerate(TILES)
            ]

            # Demand-ordered loads on the SP ring, paced so pass A's first
            # tile chases the M k-pair chunks at ~1 chunk per 0.85us of
            # compute with no PE idle gap (HAM needs ~3.4us of continuous
            # PE activity to unthrottle to 2.4 GHz): first M chunk, x tile
            # 0, rest of M's first column-half, x tiles 1-2, M's second
            # half, then the remaining x tiles (consumed 1.7us apart).
            nc.sync.dma_start(m_sb[:, 0:2, 0], m[0, 0])
            nc.sync.dma_start(x_sbs[0][:], xts[0][:])
            nc.sync.dma_start(x_sbs[1][:], xts[1][:])
            nc.sync.dma_start(x_sbs[2][:], xts[2][:])
            for g in range(1, 4):
                nc.sync.dma_start(m_sb[:, 2 * g : 2 * g + 2, 0], m[0, g])
            for t in (3, 4, 5):
                nc.sync.dma_start(x_sbs[t][:], xts[t][:])
            for g in range(4):
                nc.sync.dma_start(m_sb[:, 2 * g : 2 * g + 2, 1], m[1, g])
            for t in (6, 7):
                nc.sync.dma_start(x_sbs[t][:], xts[t][:])

            # PE warmup: garbage matmuls from a memset tile while the M
            # chunks stream in. >=3.4us of continuous PE activity = one HAM
            # SHORT window -> 2.4 GHz for the real matmuls.
            wu = wu_pool.tile([P, 256], mm_dt)
            nc.gpsimd.memset(wu[:], 0)
            ps_w = psw_pool.tile([P, 256], f32)
            for _ in range(WARMUP_MMS):
                nc.tensor.matmul(ps_w, wu[:, :128], wu[:], start=True, stop=True)

            def finish_tile(t, h, ps):
                t0, w = TILES[t]
                st = st_pool.tile([P, 512], mm_dt)
                nc.vector.tensor_copy(st[:w], ps)
                nc.scalar.dma_start(out[t0 : t0 + w, h * 512 : (h + 1) * 512], st[:w])

            def half(h, start_t):
                for t in range(start_t, len(TILES)):
                    t0, w = TILES[t]
                    if h == 1 and t == len(TILES) - 1:
                        # Final tile: two 256-col PSUM groups so the first
                        # half's cast+store overlaps the second half's
                        # matmuls, shortening the post-last-matmul tail.
                        # The very last store issues on the idle SP ring.
                        ps = ps_pool.tile([P, 512], f32, name="ps")
                        st = st_pool.tile([P, 512], mm_dt)
                        for hc in (0, 1):
                            sl = slice(256 * hc, 256 * hc + 256)
                            for k in range(KO_D):
                                nc.tensor.matmul(
                                    ps[:, sl],
                                    x_sbs[t][:, k],
                                    m_sb[:, k, h, sl],
                                    start=(k == 0),
                                    stop=(k == KO_D - 1),
                                )
                            nc.vector.tensor_copy(st[:, sl], ps[:, sl])
                            dma_eng = nc.scalar if hc == 0 else nc.sync
                            dma_eng.dma_start(
                                out[t0 : t0 + w, 512 + 256 * hc : 768 + 256 * hc],
                                st[:, sl],
                            )
                        continue
                    ps = ps_pool.tile([P, 512], f32, name="ps")[:w]
                    for k in range(KO_D):
                        nc.tensor.matmul(
                            ps,
                            x_sbs[t][:, k],
                            m_sb[:, k, h],
                            start=(k == 0),
                            stop=(k == KO_D - 1),
                        )
                    finish_tile(t, h, ps)

            # Pass A's first tiles interleave at k-pair granularity so each
            # arriving M chunk feeds 2*INTERLEAVED_TILES back-to-back
            # matmuls -- enough to cover the ~0.7-1.4us chunk interarrival
            # (jittery: the ramp saturates chip HBM across all 8 cores) and
            # keep the PE gapless while the first column-half streams in.
            ps_il = [
                ps_pool.tile([P, 512], f32, name="ps")
                for _ in range(INTERLEAVED_TILES)
            ]
            for g in range(4):
                for t in range(INTERLEAVED_TILES):
                    for k in (2 * g, 2 * g + 1):
                        nc.tensor.matmul(
                            ps_il[t],
                            x_sbs[t][:, k],
                            m_sb[:, k, 0],
                            start=(k == 0),
                            stop=(k == KO_D - 1),
                        )
            for t in range(INTERLEAVED_TILES):
                finish_tile(t, 0, ps_il[t])
            half(0, INTERLEAVED_TILES)
            half(1, 0)
    return nc


def _patch_walrus_policy():
    """Compile with walrus --policy=2 (heuristics post-scheduler, ~1.5us
    faster than --policy=0 here) and --enable-ldw-opt=true (keeps
    LDWEIGHTS double-buffered behind the previous matmul; without it each
    matmul pays a serialized ~50ns weight load: measured 263 vs 213 ns/MM)."""
    import os
    import concourse.bass_utils as bu

    if getattr(bu, "_moe_policy_patched", False):
        return
    orig = bu.run_command
    extra = [a for a in os.environ.get("MOE_WALRUS_EXTRA", "").split(",") if a]
    policy = os.environ.get("MOE_POLICY", "2")

    def _rc(argv, **kw):
        if argv and "walrus_driver" in str(argv[0]):
            argv = [f"--policy={policy}" if a == "--policy=0" else a for a in argv]
            argv = argv[:1] + ["--enable-ldw-opt=true"] + extra + argv[1:]
        return orig(argv, **kw)

    bu.run_command = _rc
    bu._moe_policy_patched = True


def _build_nc():
    import concourse.bass as bass
    import concourse.tile as tile
    from concourse import mybir

    _patch_tail_barrier(tile)
    _patch_walrus_policy()
    f32 = mybir.dt.float32
    mm_dt = mybir.dt.bfloat16
    nc = bass.Bass()
    _emit(nc, tile, mm_dt, f32)
    _legalize_waits(nc)
    return nc


_NC_CACHE = {}


def kernel(x, gate_w, gate_b, expert_w, expert_b, out_w, out_b):
    import os

    # The device path runs through the axon PJRT plugin; make sure a
    # harness-pinned JAX_PLATFORMS=cpu doesn't exclude it.
    plats = os.environ.get("JAX_PLATFORMS")
    if plats and "axon" not in plats:
        os.environ["JAX_PLATFORMS"] = plats + ",axon"

    from concourse.bass_utils import run_bass_kernel_spmd

    import ml_dtypes

    mm_np = ml_dtypes.bfloat16

    x = np.asarray(x, dtype=np.float32)
    gate_w = np.asarray(gate_w, dtype=np.float32)
    gate_b = np.asarray(gate_b, dtype=np.float32)
    expert_w = np.asarray(expert_w, dtype=np.float32)
    expert_b = np.asarray(expert_b, dtype=np.float32)
    out_w = np.asarray(out_w, dtype=np.float32)
    out_b = np.asarray(out_b, dtype=np.float32)

    xt = x.reshape(T, D)
    # Gate on host in fp64: argmax matches the fp32 reference exactly
    # (min top-2 logit gap is ~1e-5, fp64 error ~1e-12).
    logits = xt.astype(np.float64) @ gate_w.astype(np.float64) + gate_b.astype(
        np.float64
    )
    idx = np.argmax(logits, axis=1)

    # Fused per-expert weight M[e] = W[e] @ OW (exact fp32 on host), packed
    # into per-(column-half h, k-pair g) DMA chunks:
    # m[h, g, p, (kk*512+c)] = M[(2g+kk)*128+p, h*512+c]
    m_f32 = {}

    def pack_m(e):
        Me = expert_w[e] @ out_w
        m_f32[e] = Me
        return np.ascontiguousarray(
            Me.astype(mm_np)
            .reshape(4, 2, P, 2, 512)
            .transpose(3, 0, 2, 1, 4)
            .reshape(2, 4, P, 2 * 512)
        )

    tok_of_expert = [np.nonzero(idx == e)[0] for e in range(E)]
    in_maps = []
    kept = []
    overflow = []
    for e in range(E):
        toks = tok_of_expert[e]
        if len(toks) > C:
            overflow.append((e, toks[C:]))
            toks = toks[:C]
        kept.append(toks)
        xpad = np.zeros((D, C), dtype=mm_np)
        xpad[:, : len(toks)] = xt[toks].T.astype(mm_np)
        # xt{t}[p, k, j] = xpad[k*128+p, t0+j]
        xk = xpad.reshape(KO_D, P, C)
        im = {"m": pack_m(e)}
        for t, (t0, w) in enumerate(TILES):
            im[f"xt{t}"] = np.ascontiguousarray(
                xk[:, :, t0 : t0 + w].transpose(1, 0, 2)
            )
        in_maps.append(im)

    if "nc" not in _NC_CACHE:
        _NC_CACHE["nc"] = _build_nc()
    nc = _NC_CACHE["nc"]

    res = run_bass_kernel_spmd(nc, in_maps, list(range(E)))

    bias2 = (
        expert_b.astype(np.float64) @ out_w.astype(np.float64)
        + out_b.astype(np.float64)
    ).astype(np.float32)  # [E, O]

    out = np.empty((T, O), dtype=np.float32)
    for e in range(E):
        toks = kept[e]
        out[toks] = res.results[e]["out"][: len(toks)].astype(np.float32) + bias2[e]
    for e, toks in overflow:
        out[toks] = xt[toks] @ m_f32[e] + bias2[e]
    return out.reshape(B, S, O)


# revision 39
# speedup vs baseline: 1.2477x; 1.0379x over previous
"""MoE top-1 routing layer on 8 Trainium2 NeuronCores (expert-parallel).

Math: out[t] = (x[t] @ W[e] + b[e]) @ OW + ob   with e = argmax(x[t] @ GW + gb).

Both device matmuls are linear, so the host pre-fuses the weights:
  M[e]     = W[e] @ OW                    [D, O]  (exact fp32, cast bf16)
  bias2[e] = b[e] @ OW + ob               [O]     (fp64, added on host)
  out[t]   = x[t] @ M[e] + bias2[e]
so the device runs a single [C, D] @ [D, O] GEMM per core -- 1/4 the
FLOPs of the unfused two-matmul chain.

Sharding: expert-parallel. Host computes the gate (fp64 -> exact argmax),
sorts tokens by expert, pads each expert's token set to capacity C=1024
(capacity factor 1.0), and ships core e: x (gathered tokens,
transposed/striped) and M[e]. Each core returns its C token outputs
(bf16); host scatters rows back and adds bias2. Overflow tokens beyond
capacity (~3% of tokens for this routing) take an exact fp32 host path.

Device layouts (one DMA trigger per tile/chunk, 1-2 KB descriptors):
  xt{t}: [128, D/128, 128]     one per 128-token tile
  m:     [2, 4, 128, 1024]     per-(column-half h, k-pair g) chunks
  out:   [C, O] bf16
The GEMM runs in two column-half passes so the first pass only waits on
1 MB of M; the second pass is pure compute overlapped with output DMA.
A warmup matmul block keeps the PE busy through the DMA ramp so the HAM
clock gate reaches 8/8 (2.4 GHz) before the real matmuls start, and the
first pass's leading tiles interleave at k-pair granularity so the PE
chases the arriving M chunks without idle gaps.
"""

import numpy as np
from contextlib import ExitStack

B, S, D, E, H, O = 4, 2048, 1024, 8, 2048, 1024
T = B * S
P = 128
KO_D = D // P     # 8
# Per-expert device capacity = 8 full 128-token tiles (capacity factor 1.0:
# C == T/E == mean expert load). A tile's matmul cost scales with the
# 512-wide moving dim, not the token count, so a 94-token tail tile would
# cost a full tile of PE time; overflow tokens (~3.3% for the graded
# routing) take the host fallback path instead.
C = 8 * P         # 1024
TILES = [(t * P, P) for t in range(8)]

MM_DT = "bf16"    # matmul operands + device output (fp32 PSUM accumulation)


def _legalize_waits(nc):
    """This container's walrus accepts 1 sem wait per instruction (2 for
    EventSemaphore); Tile's tail drain can carry more. Split the excess
    onto preceding same-engine NoOps."""
    from concourse import mybir

    uid = 0
    for f in nc.m.functions:
        for b in f.blocks:
            insts = b.instructions
            out = []
            changed = False
            for ins in insts:
                si = ins.sync_info
                waits = list(si.on_wait) if si is not None else []
                limit = 2 if str(ins.opcode) == "EventSemaphore" else 1
                if len(waits) > limit:
                    extra, keep = waits[:-limit], waits[-limit:]
                    for w in extra:
                        uid += 1
                        out.append(
                            mybir.InstNoOp(
                                name=f"waitsplit-{uid}",
                                engine=ins.engine,
                                sync_info=mybir.SyncInfo(on_wait=[w], on_update=[]),
                                bass_nofuse=True,
                            )
                        )
                    si.on_wait = keep
                    changed = True
                out.append(ins)
            if changed:
                insts.clear()
                insts.extend(out)


def _patch_tail_barrier(tile_mod):
    """Tile's kernel tail is drain -> barrier -> sem-reset -> barrier.
    The second all-engine barrier only orders the sem-reset against program
    end, which the per-engine stream end already guarantees; drop it."""
    if getattr(tile_mod.TileContext, "_moe_tail_patched", False):
        return
    from concourse.vector_clock import ScopedClock

    def _drain_and_barrier(self, tick_clock, wait_clock):
        drain_inst = self.nc.sync.drain()
        wait_clock.add_sem_waits(
            drain_inst.ins, ScopedClock({None: tick_clock.global_clock})
        )
        self.nc.all_engine_barrier()
        popped = self.nc._tile_sem_poison_stack.pop()
        assert popped is self._sem_poison
        self.nc.clear_and_free_semaphores(list(self.sems.allocated().values()))

    tile_mod.TileContext._drain_and_barrier = _drain_and_barrier
    tile_mod.TileContext._moe_tail_patched = True


WARMUP_MMS = 19
INTERLEAVED_TILES = 3  # pass-A tiles interleaved at k-pair granularity


def _emit(nc, tile, mm_dt, f32):
    """Single fused GEMM out = x @ M, x and M SBUF-resident.

    Input loads issue on the SP HWDGE ring (nc.sync), output stores on the
    ACT ring (nc.scalar): DMA_DIRECT2D costs ~0.6us of issue time on its
    sequencer, so keeping the 18 stores off the input FIFO stops them
    head-of-line blocking the x/M stream. A memset-fed warmup matmul block
    keeps the PE busy while the first M chunks stream in, flipping the HAM
    clock gate to 8/8 (2.4 GHz) before the real matmuls start.
    """
    xts = [
        nc.dram_tensor(f"xt{t}", [P, KO_D, w], mm_dt, kind="ExternalInput")
        for t, (t0, w) in enumerate(TILES)
    ]
    # m[h, g] = one DMA chunk: column-half h, k-pair g (2 k-tiles, 256 KB)
    m = nc.dram_tensor("m", [2, 4, P, 2 * 512], mm_dt, kind="ExternalInput")
    out = nc.dram_tensor("out", [C, O], mm_dt, kind="ExternalOutput")

    with tile.TileContext(nc) as tc:
        with ExitStack() as ctx:
            x_pool = ctx.enter_context(tc.tile_pool(name="x", bufs=1))
            m_pool = ctx.enter_context(tc.tile_pool(name="m", bufs=1))
            wu_pool = ctx.enter_context(tc.tile_pool(name="wu", bufs=1))
            st_pool = ctx.enter_context(tc.tile_pool(name="st", bufs=3))
            ps_pool = ctx.enter_context(
                tc.tile_pool(name="ps", bufs=4, space="PSUM")
            )
            psw_pool = ctx.enter_context(
                tc.tile_pool(name="psw", bufs=1, space="PSUM")
            )

            m_sb = m_pool.tile([P, KO_D, 2, 512], mm_dt)
            x_sbs = [
                x_pool.tile([P, KO_D, w], mm_dt, name=f"x{t}")
                for t, (t0, w) in enumerate(TILES)
            ]

            # Demand-ordered loads on the SP ring, paced so pass A's first
            # tile chases the M k-pair chunks at ~1 chunk per 0.85us of
            # compute with no PE idle gap (HAM needs ~3.4us of continuous
            # PE activity to unthrottle to 2.4 GHz): first M chunk, x tile
            # 0, rest of M's first column-half, x tiles 1-2, M's second
            # half, then the remaining x tiles (consumed 1.7us apart).
            nc.sync.dma_start(m_sb[:, 0:2, 0], m[0, 0])
            nc.sync.dma_start(x_sbs[0][:], xts[0][:])
            nc.sync.dma_start(x_sbs[1][:], xts[1][:])
            nc.sync.dma_start(x_sbs[2][:], xts[2][:])
            for g in range(1, 4):
                nc.sync.dma_start(m_sb[:, 2 * g : 2 * g + 2, 0], m[0, g])
            for t in (3, 4, 5):
                nc.sync.dma_start(x_sbs[t][:], xts[t][:])
            for g in range(4):
                nc.sync.dma_start(m_sb[:, 2 * g : 2 * g + 2, 1], m[1, g])
            for t in (6, 7):
                nc.sync.dma_start(x_sbs[t][:], xts[t][:])

            # PE warmup: garbage matmuls from a memset tile while the M
            # chunks stream in. >=3.4us of continuous PE activity = one HAM
            # SHORT window -> 2.4 GHz for the real matmuls.
            wu = wu_pool.tile([P, 256], mm_dt)
            nc.gpsimd.memset(wu[:], 0)
            ps_w = psw_pool.tile([P, 256], f32)
            for _ in range(WARMUP_MMS):
                nc.tensor.matmul(ps_w, wu[:, :128], wu[:], start=True, stop=True)

            def finish_tile(t, h, ps):
                t0, w = TILES[t]
                st = st_pool.tile([P, 512], mm_dt)
                nc.vector.tensor_copy(st[:w], ps)
                nc.scalar.dma_start(out[t0 : t0 + w, h * 512 : (h + 1) * 512], st[:w])

            def half(h, start_t):
                for t in range(start_t, len(TILES)):
                    t0, w = TILES[t]
                    if h == 1 and t == len(TILES) - 1:
                        # Final tile: two 256-col PSUM groups so the first
                        # half's cast+store overlaps the second half's
                        # matmuls, shortening the post-last-matmul tail.
                        # Group b accumulates in the warmup bank (psw, free
                        # since the ramp): sharing one ps tile would make
                        # Tile serialize group b's matmuls behind group a's
                        # cast (write-after-read), a ~0.7us stall. The very
                        # last store issues on the idle SP ring.
                        ps_a = ps_pool.tile([P, 512], f32, name="ps")[:, :256]
                        ps_b = psw_pool.tile([P, 256], f32)
                        st = st_pool.tile([P, 512], mm_dt)
                        for hc, ps in ((0, ps_a), (1, ps_b)):
                            sl = slice(256 * hc, 256 * hc + 256)
                            for k in range(KO_D):
                                nc.tensor.matmul(
                                    ps,
                                    x_sbs[t][:, k],
                                    m_sb[:, k, h, sl],
                                    start=(k == 0),
                                    stop=(k == KO_D - 1),
                                )
                            nc.vector.tensor_copy(st[:, sl], ps)
                            dma_eng = nc.scalar if hc == 0 else nc.sync
                            dma_eng.dma_start(
                                out[t0 : t0 + w, 512 + 256 * hc : 768 + 256 * hc],
                                st[:, sl],
                            )
                        continue
                    ps = ps_pool.tile([P, 512], f32, name="ps")[:w]
                    for k in range(KO_D):
                        nc.tensor.matmul(
                            ps,
                            x_sbs[t][:, k],
                            m_sb[:, k, h],
                            start=(k == 0),
                            stop=(k == KO_D - 1),
                        )
                    finish_tile(t, h, ps)

            # Pass A's first tiles interleave at k-pair granularity so each
            # arriving M chunk feeds 2*INTERLEAVED_TILES back-to-back
            # matmuls -- enough to cover the ~0.7-1.4us chunk interarrival
            # (jittery: the ramp saturates chip HBM across all 8 cores) and
            # keep the PE gapless while the first column-half streams in.
            ps_il = [
                ps_pool.tile([P, 512], f32, name="ps")
                for _ in range(INTERLEAVED_TILES)
            ]
            for g in range(4):
                for t in range(INTERLEAVED_TILES):
                    for k in (2 * g, 2 * g + 1):
                        nc.tensor.matmul(
                            ps_il[t],
                            x_sbs[t][:, k],
                            m_sb[:, k, 0],
                            start=(k == 0),
                            stop=(k == KO_D - 1),
                        )
            for t in range(INTERLEAVED_TILES):
                finish_tile(t, 0, ps_il[t])
            half(0, INTERLEAVED_TILES)
            half(1, 0)
    return nc


def _patch_walrus_policy():
    """Compile with walrus --policy=2 (heuristics post-scheduler, ~1.5us
    faster than --policy=0 here) and --enable-ldw-opt=true (keeps
    LDWEIGHTS double-buffered behind the previous matmul; without it each
    matmul pays a serialized ~50ns weight load: measured 263 vs 213 ns/MM)."""
    import os
    import concourse.bass_utils as bu

    if getattr(bu, "_moe_policy_patched", False):
        return
    orig = bu.run_command
    extra = [a for a in os.environ.get("MOE_WALRUS_EXTRA", "").split(",") if a]
    policy = os.environ.get("MOE_POLICY", "2")

    def _rc(argv, **kw):
        if argv and "walrus_driver" in str(argv[0]):
            argv = [f"--policy={policy}" if a == "--policy=0" else a for a in argv]
            argv = argv[:1] + ["--enable-ldw-opt=true"] + extra + argv[1:]
        return orig(argv, **kw)

    bu.run_command = _rc
    bu._moe_policy_patched = True


def _build_nc():
    import concourse.bass as bass
    import concourse.tile as tile
    from concourse import mybir

    _patch_tail_barrier(tile)
    _patch_walrus_policy()
    f32 = mybir.dt.float32
    mm_dt = mybir.dt.bfloat16
    nc = bass.Bass()
    _emit(nc, tile, mm_dt, f32)
    _legalize_waits(nc)
    return nc


_NC_CACHE = {}


def kernel(x, gate_w, gate_b, expert_w, expert_b, out_w, out_b):
    import os

    # The device path runs through the axon PJRT plugin; make sure a
    # harness-pinned JAX_PLATFORMS=cpu doesn't exclude it.
    plats = os.environ.get("JAX_PLATFORMS")
    if plats and "axon" not in plats:
        os.environ["JAX_PLATFORMS"] = plats + ",axon"

    from concourse.bass_utils import run_bass_kernel_spmd

    import ml_dtypes

    mm_np = ml_dtypes.bfloat16

    x = np.asarray(x, dtype=np.float32)
    gate_w = np.asarray(gate_w, dtype=np.float32)
    gate_b = np.asarray(gate_b, dtype=np.float32)
    expert_w = np.asarray(expert_w, dtype=np.float32)
    expert_b = np.asarray(expert_b, dtype=np.float32)
    out_w = np.asarray(out_w, dtype=np.float32)
    out_b = np.asarray(out_b, dtype=np.float32)

    xt = x.reshape(T, D)
    # Gate on host in fp64: argmax matches the fp32 reference exactly
    # (min top-2 logit gap is ~1e-5, fp64 error ~1e-12).
    logits = xt.astype(np.float64) @ gate_w.astype(np.float64) + gate_b.astype(
        np.float64
    )
    idx = np.argmax(logits, axis=1)

    # Fused per-expert weight M[e] = W[e] @ OW (exact fp32 on host), packed
    # into per-(column-half h, k-pair g) DMA chunks:
    # m[h, g, p, (kk*512+c)] = M[(2g+kk)*128+p, h*512+c]
    m_f32 = {}

    def pack_m(e):
        Me = expert_w[e] @ out_w
        m_f32[e] = Me
        return np.ascontiguousarray(
            Me.astype(mm_np)
            .reshape(4, 2, P, 2, 512)
            .transpose(3, 0, 2, 1, 4)
            .reshape(2, 4, P, 2 * 512)
        )

    tok_of_expert = [np.nonzero(idx == e)[0] for e in range(E)]
    in_maps = []
    kept = []
    overflow = []
    for e in range(E):
        toks = tok_of_expert[e]
        if len(toks) > C:
            overflow.append((e, toks[C:]))
            toks = toks[:C]
        kept.append(toks)
        xpad = np.zeros((D, C), dtype=mm_np)
        xpad[:, : len(toks)] = xt[toks].T.astype(mm_np)
        # xt{t}[p, k, j] = xpad[k*128+p, t0+j]
        xk = xpad.reshape(KO_D, P, C)
        im = {"m": pack_m(e)}
        for t, (t0, w) in enumerate(TILES):
            im[f"xt{t}"] = np.ascontiguousarray(
                xk[:, :, t0 : t0 + w].transpose(1, 0, 2)
            )
        in_maps.append(im)

    if "nc" not in _NC_CACHE:
        _NC_CACHE["nc"] = _build_nc()
    nc = _NC_CACHE["nc"]

    res = run_bass_kernel_spmd(nc, in_maps, list(range(E)))

    bias2 = (
        expert_b.astype(np.float64) @ out_w.astype(np.float64)
        + out_b.astype(np.float64)
    ).astype(np.float32)  # [E, O]

    out = np.empty((T, O), dtype=np.float32)
    for e in range(E):
        toks = kept[e]
        out[toks] = res.results[e]["out"][: len(toks)].astype(np.float32) + bias2[e]
    for e, toks in overflow:
        out[toks] = xt[toks] @ m_f32[e] + bias2[e]
    return out.reshape(B, S, O)
